# revision 13
# baseline (speedup 1.0000x reference)
"""Sliding-window (chunked) multi-head attention for Trainium2, 8-core SPMD.

Problem: B=1, S=8192, E=512, H=8 heads, Dh=64, window=1024 (half=512).
Reference math per window i (size 1024): keys span [i-512, i+1536).

Sharding: core c owns query window [1024c, 1024c+1024); it receives
x^T for the halo'd key range [1024c-512, 1024c+1536) (zero-padded at
the sequence edges) and computes q/k/v projections locally, windowed
softmax(q k^T / 8) v, and the output projection.  All compute layouts
are transposed ([E, seq]) so every matmul contracts over partitions;
the softmax denominator comes from a ones-augmented v (65th column).
bv is folded into an adjusted output-projection bias on the host
(attn rows sum to 1), so v needs no bias add on-chip.

Pipeline structure (v2):
 - x is DMA'd in four 512-column sequence chunks on four DMA rings so
   the v projection starts ~1.5us in.
 - PSUM: 4x [128,512] score tiles (1 bank each) + 2x [65,1024] AV
   accumulators (2 banks each) = 8 banks, double-buffered end to end.
 - exp split: head0 tiles on Vector (custom cubic^4 DVE op), head1
   tiles on Scalar (Act exp).  All PSUM evacuations live on V/S
   (GpSimd has no PSUM port on TRN2); GpSimd does the SBUF-side
   normalize via tensor_tensor divide on a DMA-broadcast denominator.
 - score matmuls for the two heads are issued interleaved so their
   K=64 matmuls row-pack into disjoint halves of the PE array.

Outputs are y^T shards [512, 1024] per core; the host transposes and
concatenates.
"""

import numpy as np
import ml_dtypes

import concourse.bass as bass
import concourse.tile as tile
from concourse import bacc, mybir
from concourse import bass_utils
from concourse.bass import ts

# ---- problem constants (hardcoded per contract) ----
S = 8192
E = 512
H = 8
DH = 64
NCORES = 8
SQ = 1024          # queries per core
SK = 2048          # halo'd keys per core
HALF = 512
SCALE = 0.125      # 1/sqrt(64)

F32 = mybir.dt.float32
F32R = mybir.dt.float32r
BF16 = mybir.dt.bfloat16
FP16 = mybir.dt.float16

# ---- custom DVE op: exp(u/8) ~= (1 + c1 u + c2 u^2 + c3 u^3)^4 ----
# Fitted (Lawson minimax) on |u/8| <= 1.6; max rel err 7.2e-4.
_EC1 = 0.03126080224663743
_EC2 = 0.000493647595612354
_EC3 = 5.0261583805949835e-06


def _register_exp_op():
    from concourse import dve_ops as dops
    from concourse.dve_spec import Spec, Src0, One, C0, C1, C2, sq, lower
    from concourse.dve_uop import DveOpSpec

    name = "EXP4_ANT"
    for op in dops.OPS:
        if op.name == name:
            return op
    body = sq(sq(((C2 * Src0 + C1) * Src0 + C0) * Src0 + One))
    spec = Spec(body=body)
    shas = {}
    for ver in ("v3", "v4"):
        uops = lower(spec, ver=ver)
        shas[ver] = DveOpSpec(name=name, opcode=0, uops=uops, rd1_en=False).sha(ver)
    op = dops.DveOp(name, spec, subdim=False, uops_sha=shas)
    dops.OPS.append(op)
    dops.CUSTOM_DVE_SPECS[name] = spec
    dops._SUB_OPCODE_FOR_NAME[name] = dops._CUSTOM_DVE_ROW_BASE + len(dops.OPS) - 1
    assert max(dops._SUB_OPCODE_FOR_NAME.values()) < 0x20
    return op


def _build():
    """Build + compile the per-core Bass program (SPMD: same NEFF, 8 cores)."""
    exp_op = _register_exp_op()

    nc = bacc.Bacc("TRN2", target_bir_lowering=False, debug=False)

    xT_d = nc.dram_tensor("xT", [E, SK], FP16, kind="ExternalInput")
    W_d = {
        n: nc.dram_tensor(n, [128, E // 128, E], FP16, kind="ExternalInput")
        for n in ("Wq", "Wk", "Wv", "Wo")
    }
    bq_d = nc.dram_tensor("bq", [E], F32, kind="ExternalInput")
    bk_d = nc.dram_tensor("bk", [E], F32, kind="ExternalInput")
    bo_d = nc.dram_tensor("bo_eff", [E], F32, kind="ExternalInput")
    mask_d = nc.dram_tensor("mask8", [128, SK // 128, H], FP16, kind="ExternalInput")
    yT_d = nc.dram_tensor("yT", [E, SQ], FP16, kind="ExternalOutput")

    KT = 4           # E // 128 contraction tiles
    NKT = SK // 128  # 16 key tiles

    with tile.TileContext(nc) as tc:
        with (
            nc.allow_low_precision(reason="fp16/f32r attention kernel"),
            tc.tile_pool(name="singles", bufs=1) as singles,
            tc.tile_pool(name="exps", bufs=8) as exps,
            tc.tile_pool(name="avus", bufs=3) as avus,
            tc.tile_pool(name="bcs", bufs=4) as bcs,
            tc.tile_pool(name="dscratch", bufs=2, space="DRAM") as dscratch,
            tc.tile_pool(name="ystage", bufs=3) as ystage,
        ):
            # ---- input DMAs: x in 4 seq-chunks on 4 rings ----
            W_sb = {}
            for n in ("Wq", "Wk", "Wv", "Wo"):
                W_sb[n] = singles.tile([128, KT, E], FP16, tag=f"w_{n}", name=f"w_{n}")
            xT_sb = singles.tile([128, KT, SK], FP16, tag="xT")

            xin = xT_d.ap().rearrange("(t p) s -> p t s", p=128)
            nc.sync.dma_start(out=W_sb["Wv"], in_=W_d["Wv"].ap())
            nc.scalar.dma_start(out=xT_sb[:, :, ts(0, 512)], in_=xin[:, :, ts(0, 512)])
            nc.gpsimd.dma_start(out=xT_sb[:, :, ts(1, 512)], in_=xin[:, :, ts(1, 512)])
            nc.sync.dma_start(out=xT_sb[:, :, ts(2, 512)], in_=xin[:, :, ts(2, 512)])
            nc.scalar.dma_start(out=xT_sb[:, :, ts(3, 512)], in_=xin[:, :, ts(3, 512)])
            nc.gpsimd.dma_start(out=W_sb["Wq"], in_=W_d["Wq"].ap())
            nc.scalar.dma_start(out=W_sb["Wk"], in_=W_d["Wk"].ap())
            nc.gpsimd.dma_start(out=W_sb["Wo"], in_=W_d["Wo"].ap())

            bq_sb = singles.tile([128, KT], F32, tag="bq")
            nc.sync.dma_start(out=bq_sb, in_=bq_d.ap().rearrange("(t p) -> p t", p=128))
            bk_sb = singles.tile([128, KT], F32, tag="bk")
            nc.sync.dma_start(out=bk_sb, in_=bk_d.ap().rearrange("(t p) -> p t", p=128))
            bo_sb = singles.tile([128, KT], F32, tag="bo")
            nc.sync.dma_start(out=bo_sb, in_=bo_d.ap().rearrange("(t p) -> p t", p=128))

            # v with ones column (from mask: 0 for padded keys)
            v_sb = singles.tile([128, NKT, H, DH + 1], FP16, tag="v")
            nc.sync.dma_start(out=v_sb[:, :, :, DH], in_=mask_d.ap())

            qT_sb = singles.tile([128, KT, SQ], FP16, tag="qT")
            kT_sb = singles.tile([128, KT, SK], FP16, tag="kT")
            outT_sb = singles.tile([128, KT, SQ], FP16, tag="outT")

            # preload the Act exp table set (2.7us) before it's needed
            warm = singles.tile([1, 16], F32, tag="warm")
            nc.vector.memset(warm, 0.0)
            warm2 = singles.tile([1, 16], F32, tag="warm2")
            nc.scalar.activation(out=warm2, in_=warm,
                                 func=mybir.ActivationFunctionType.Exp)

            # ---- projections ----
            with tc.tile_pool(name="pproj", bufs=4, space="PSUM") as pproj:
                # v projection: per 128-key tile st, needs x chunk st//4 only
                for st in range(NKT):
                    ps = pproj.tile([128, 512], F32, tag="pp")
                    for ke in range(KT):
                        nc.tensor.matmul(
                            ps,
                            xT_sb[:, ke, ts(st, 128)],
                            W_sb["Wv"][:, ke, :],
                            start=(ke == 0), stop=(ke == KT - 1),
                        )
                    nc.scalar.activation(
                        out=v_sb[:, st, :, 0:DH],
                        in_=ps.rearrange("p (h d) -> p h d", h=H),
                        func=mybir.ActivationFunctionType.Copy,
                    )
                # k projection: ke-outer so each W tile is LDW'd once
                for th in range(KT):
                    pks = [pproj.tile([128, 512], F32, tag="pp", name=f"pk{kc}")
                           for kc in range(4)]
                    for ke in range(KT):
                        for kc in range(4):
                            nc.tensor.matmul(
                                pks[kc],
                                W_sb["Wk"][:, ke, ts(th, 128)],
                                xT_sb[:, ke, ts(kc, 512)],
                                start=(ke == 0), stop=(ke == KT - 1),
                            )
                    for kc in range(4):
                        nc.vector.tensor_scalar_add(
                            out=kT_sb[:, th, ts(kc, 512)], in0=pks[kc],
                            scalar1=bk_sb[:, th:th + 1],
                        )
                # q projection
                for th in range(KT):
                    pqs = [pproj.tile([128, 512], F32, tag="pp", name=f"pq{qc}")
                           for qc in range(2)]
                    for ke in range(KT):
                        for qc in range(2):
                            nc.tensor.matmul(
                                pqs[qc],
                                W_sb["Wq"][:, ke, ts(th, 128)],
                                xT_sb[:, ke, HALF + qc * 512:HALF + (qc + 1) * 512],
                                start=(ke == 0), stop=(ke == KT - 1),
                            )
                    for qc in range(2):
                        nc.vector.tensor_scalar_add(
                            out=qT_sb[:, th, ts(qc, 512)], in0=pqs[qc],
                            scalar1=bq_sb[:, th:th + 1],
                        )

            # ---- windowed attention, two heads (one head-pair) at a time ----
            with (
                tc.tile_pool(name="pscore", bufs=4, space="PSUM") as pscore,
                tc.tile_pool(name="pav", bufs=2, space="PSUM") as pav,
            ):
                for hp in range(H // 2):
                    th = hp
                    avps = []
                    for i in range(2):
                        av_i = pav.tile([DH + 1, SQ], F32, tag="av", name=f"av{i}")
                        avps.append(av_i)
                    for kt in range(NKT):
                        # scores: 4x [128,512] tiles; heads interleaved so the
                        # two K=64 matmuls row-pack into array halves
                        s_t = {}
                        for qc in range(2):
                            for i in range(2):
                                r0 = 64 * i
                                sp = pscore.tile([128, 512], F32, tag="s",
                                                 name=f"s{i}q{qc}")
                                nc.tensor.matmul(
                                    sp,
                                    kT_sb[r0:r0 + 64, th, ts(kt, 128)],
                                    qT_sb[r0:r0 + 64, th, ts(qc, 512)],
                                    start=True, stop=True,
                                )
                                s_t[(i, qc)] = sp
                        e_t = {}
                        for qc in range(2):
                            e0 = exps.tile([128, 512], FP16, tag="e", name=f"e0q{qc}")
                            nc.vector._custom_dve(
                                exp_op, out=e0, in0=s_t[(0, qc)],
                                s0=_EC1, s1=_EC2, imm2=_EC3,
                            )
                            e_t[(0, qc)] = e0
                            e1 = exps.tile([128, 512], FP16, tag="e", name=f"e1q{qc}")
                            nc.scalar.activation(
                                out=e1, in_=s_t[(1, qc)],
                                func=mybir.ActivationFunctionType.Exp, scale=SCALE,
                            )
                            e_t[(1, qc)] = e1
                        for qc in range(2):
                            for i in range(2):
                                h = 2 * hp + i
                                nc.tensor.matmul(
                                    avps[i][:, ts(qc, 512)],
                                    v_sb[:, kt, h, :],
                                    e_t[(i, qc)],
                                    start=(kt == 0), stop=(kt == NKT - 1),
                                )
                    # finalize: evacuate accumulators, then normalize on the
                    # SBUF side (GpSimd divide by a DMA-broadcast denominator)
                    for i in range(2):
                        h = 2 * hp + i
                        r0 = 64 * i
                        avu = avus.tile([DH + 1, SQ], F32, tag="avu")
                        nc.scalar.activation(out=avu, in_=avps[i],
                                             func=mybir.ActivationFunctionType.Copy)
                        # denominator row -> DRAM -> stride-0 partition
                        # broadcast, then a vector divide (V has slack in the
                        # tensor-bound steady state); 2 DMA hops instead of 4.
                        d_dram = dscratch.tile([1, SQ], F32, tag="dd")
                        nc.sync.dma_start(out=d_dram, in_=avu[DH:DH + 1, :])
                        bc = bcs.tile([DH, SQ], F32, tag="bc")
                        nc.sync.dma_start(
                            out=bc,
                            in_=bass.AP(
                                tensor=d_dram.tensor,
                                offset=d_dram.offset,
                                ap=[[0, DH]] + [list(a) for a in d_dram.ap[1:]],
                            ),
                        )
                        rb = bcs.tile([DH, SQ], F32, tag="rb")
                        nc.vector.reciprocal(out=rb, in_=bc)
                        for qc in range(2):
                            nc.vector.tensor_mul(
                                out=outT_sb[r0:r0 + 64, th, ts(qc, 512)],
                                in0=avu[0:DH, ts(qc, 512)],
                                in1=rb[:, ts(qc, 512)],
                            )

            # ---- output projection ----
            with tc.tile_pool(name="py", bufs=4, space="PSUM") as py:
                for m in range(KT):
                    pys = [py.tile([128, 512], F32, tag="py", name=f"py{qc}")
                           for qc in range(2)]
                    for ke in range(KT):
                        for qc in range(2):
                            nc.tensor.matmul(
                                pys[qc],
                                W_sb["Wo"][:, ke, ts(m, 128)],
                                outT_sb[:, ke, ts(qc, 512)],
                                start=(ke == 0), stop=(ke == KT - 1),
                            )
                    for qc in range(2):
                        yst = ystage.tile([128, 512], FP16, tag="y")
                        nc.vector.tensor_scalar_add(out=yst, in0=pys[qc],
                                                    scalar1=bo_sb[:, m:m + 1])
                        nc.sync.dma_start(out=yT_d[ts(m, 128), ts(qc, 512)], in_=yst)

    nc.compile()
    return nc


_NC_CACHE = []


def _get_nc():
    if not _NC_CACHE:
        _NC_CACHE.append(_build())
    return _NC_CACHE[0]


def _prep_inputs(x, Wq, bq, Wk, bk, Wv, bv, Wo, bo):
    x = np.asarray(x, np.float32)
    xT_full = np.ascontiguousarray(x[0].T)  # [E, S]
    bo_eff = (np.asarray(bo, np.float64)
              + np.asarray(bv, np.float64) @ np.asarray(Wo, np.float64)).astype(np.float32)
    def wprep(W):
        Wb = np.asarray(W, np.float32).astype(np.float16)
        return np.ascontiguousarray(Wb.reshape(4, 128, E).transpose(1, 0, 2))

    shared = {
        "Wq": wprep(Wq),
        "Wk": wprep(Wk),
        "Wv": wprep(Wv),
        "Wo": wprep(Wo),
        "bq": np.asarray(bq, np.float32),
        "bk": np.asarray(bk, np.float32),
        "bo_eff": bo_eff,
    }
    in_maps = []
    for c in range(NCORES):
        g0 = 1024 * c - HALF
        xT_halo = np.zeros((E, SK), np.float32)
        lo, hi = max(0, g0), min(S, g0 + SK)
        xT_halo[:, lo - g0:hi - g0] = xT_full[:, lo:hi]
        mask = np.zeros((SK, H), np.float32)
        mask[lo - g0:hi - g0, :] = 1.0
        mask = np.ascontiguousarray(mask.reshape(SK // 128, 128, H).transpose(1, 0, 2))
        m = dict(shared)
        m["xT"] = xT_halo.astype(np.float16)
        m["mask8"] = mask.astype(np.float16)
        in_maps.append(m)
    return in_maps


def run(inputs: dict, trace: bool = False):
    nc = _get_nc()
    in_maps = _prep_inputs(**inputs)
    res = bass_utils.run_bass_kernel_spmd(
        nc, in_maps, core_ids=list(range(NCORES)), trace=trace
    )
    y = np.concatenate([r["yT"].T for r in res.results], axis=0)[None]
    return np.ascontiguousarray(y.astype(np.float32)), res


def kernel(**inputs) -> np.ndarray:
    y, _ = run(inputs, trace=False)
    return y


# revision 14
# speedup vs baseline: 1.2850x; 1.2850x over previous
"""Sliding-window (chunked) multi-head attention for Trainium2, 8-core SPMD.

Problem: B=1, S=8192, E=512, H=8 heads, Dh=64, window=1024 (half=512).
Reference math per window i (size 1024): keys span [i-512, i+1536).

Sharding: core c owns query window [1024c, 1024c+1024); it receives
x^T for the halo'd key range [1024c-512, 1024c+1536) (zero-padded at
the sequence edges) and computes q/k/v projections locally, windowed
softmax(q k^T / 8) v, and the output projection.  All compute layouts
are transposed ([E, seq]) so every matmul contracts over partitions;
the softmax denominator comes from a ones-augmented v (65th column).
bv is folded into an adjusted output-projection bias on the host
(attn rows sum to 1), so v needs no bias add on-chip.

Pipeline structure (v2):
 - x is DMA'd in four 512-column sequence chunks on four DMA rings so
   the v projection starts ~1.5us in.
 - PSUM: 4x [128,512] score tiles (1 bank each) + 2x [65,1024] AV
   accumulators (2 banks each) = 8 banks, double-buffered end to end.
 - exp split: head0 tiles on Vector (custom cubic^4 DVE op), head1
   tiles on Scalar (Act exp).  All PSUM evacuations live on V/S
   (GpSimd has no PSUM port on TRN2); GpSimd does the SBUF-side
   normalize via tensor_tensor divide on a DMA-broadcast denominator.
 - score matmuls for the two heads are issued interleaved so their
   K=64 matmuls row-pack into disjoint halves of the PE array.

Outputs are y^T shards [512, 1024] per core; the host transposes and
concatenates.
"""

import numpy as np
import ml_dtypes

import concourse.bass as bass
import concourse.tile as tile
from concourse import bacc, mybir
from concourse import bass_utils
from concourse.bass import ts

# ---- problem constants (hardcoded per contract) ----
S = 8192
E = 512
H = 8
DH = 64
NCORES = 8
SQ = 1024          # queries per core
SK = 2048          # halo'd keys per core
HALF = 512
SCALE = 0.125      # 1/sqrt(64)

F32 = mybir.dt.float32
F32R = mybir.dt.float32r
BF16 = mybir.dt.bfloat16
FP16 = mybir.dt.float16

# ---- custom DVE op: exp(u/8) ~= (1 + c1 u + c2 u^2 + c3 u^3)^4 ----
# Fitted (Lawson minimax) on |u/8| <= 1.6; max rel err 7.2e-4.
_EC1 = 0.03126080224663743
_EC2 = 0.000493647595612354
_EC3 = 5.0261583805949835e-06


def _register_exp_op():
    from concourse import dve_ops as dops
    from concourse.dve_spec import Spec, Src0, One, C0, C1, C2, sq, lower
    from concourse.dve_uop import DveOpSpec

    name = "EXP4_ANT"
    for op in dops.OPS:
        if op.name == name:
            return op
    body = sq(sq(((C2 * Src0 + C1) * Src0 + C0) * Src0 + One))
    spec = Spec(body=body)
    shas = {}
    for ver in ("v3", "v4"):
        uops = lower(spec, ver=ver)
        shas[ver] = DveOpSpec(name=name, opcode=0, uops=uops, rd1_en=False).sha(ver)
    op = dops.DveOp(name, spec, subdim=False, uops_sha=shas)
    dops.OPS.append(op)
    dops.CUSTOM_DVE_SPECS[name] = spec
    dops._SUB_OPCODE_FOR_NAME[name] = dops._CUSTOM_DVE_ROW_BASE + len(dops.OPS) - 1
    assert max(dops._SUB_OPCODE_FOR_NAME.values()) < 0x20
    return op


def _build():
    """Build + compile the per-core Bass program (SPMD: same NEFF, 8 cores)."""
    exp_op = _register_exp_op()

    nc = bacc.Bacc("TRN2", target_bir_lowering=False, debug=False)

    xT_d = nc.dram_tensor("xT", [E, SK], FP16, kind="ExternalInput")
    W_d = {
        n: nc.dram_tensor(n, [128, E // 128, E], FP16, kind="ExternalInput")
        for n in ("Wq", "Wk", "Wv", "Wo")
    }
    bq_d = nc.dram_tensor("bq", [E], F32, kind="ExternalInput")
    bk_d = nc.dram_tensor("bk", [E], F32, kind="ExternalInput")
    bo_d = nc.dram_tensor("bo_eff", [E], F32, kind="ExternalInput")
    mask_d = nc.dram_tensor("mask8", [128, SK // 128, H], FP16, kind="ExternalInput")
    yT_d = nc.dram_tensor("yT", [E, SQ], FP16, kind="ExternalOutput")

    KT = 4           # E // 128 contraction tiles
    NKT = SK // 128  # 16 key tiles

    with tile.TileContext(nc) as tc:
        with (
            nc.allow_low_precision(reason="fp16/f32r attention kernel"),
            tc.tile_pool(name="singles", bufs=1) as singles,
            tc.tile_pool(name="exps", bufs=8) as exps,
            tc.tile_pool(name="avus", bufs=3) as avus,
            tc.tile_pool(name="bcs", bufs=4) as bcs,
            tc.tile_pool(name="dscratch", bufs=2, space="DRAM") as dscratch,
            tc.tile_pool(name="ystage", bufs=3) as ystage,
        ):
            # ---- input DMAs: x in 4 seq-chunks on 4 rings ----
            W_sb = {}
            for n in ("Wq", "Wk", "Wv", "Wo"):
                W_sb[n] = singles.tile([128, KT, E], FP16, tag=f"w_{n}", name=f"w_{n}")
            xT_sb = singles.tile([128, KT, SK], FP16, tag="xT")

            xin = xT_d.ap().rearrange("(t p) s -> p t s", p=128)
            nc.sync.dma_start(out=W_sb["Wv"], in_=W_d["Wv"].ap())
            nc.scalar.dma_start(out=xT_sb[:, :, ts(0, 512)], in_=xin[:, :, ts(0, 512)])
            nc.gpsimd.dma_start(out=xT_sb[:, :, ts(1, 512)], in_=xin[:, :, ts(1, 512)])
            nc.sync.dma_start(out=xT_sb[:, :, ts(2, 512)], in_=xin[:, :, ts(2, 512)])
            nc.scalar.dma_start(out=xT_sb[:, :, ts(3, 512)], in_=xin[:, :, ts(3, 512)])
            nc.gpsimd.dma_start(out=W_sb["Wq"], in_=W_d["Wq"].ap())
            nc.scalar.dma_start(out=W_sb["Wk"], in_=W_d["Wk"].ap())
            nc.gpsimd.dma_start(out=W_sb["Wo"], in_=W_d["Wo"].ap())

            bq_sb = singles.tile([128, KT], F32, tag="bq")
            nc.sync.dma_start(out=bq_sb, in_=bq_d.ap().rearrange("(t p) -> p t", p=128))
            bk_sb = singles.tile([128, KT], F32, tag="bk")
            nc.sync.dma_start(out=bk_sb, in_=bk_d.ap().rearrange("(t p) -> p t", p=128))
            bo_sb = singles.tile([128, KT], F32, tag="bo")
            nc.sync.dma_start(out=bo_sb, in_=bo_d.ap().rearrange("(t p) -> p t", p=128))

            # v with ones column (from mask: 0 for padded keys)
            v_sb = singles.tile([128, NKT, H, DH + 1], FP16, tag="v")
            nc.sync.dma_start(out=v_sb[:, :, :, DH], in_=mask_d.ap())

            qT_sb = singles.tile([128, KT, SQ], FP16, tag="qT")
            kT_sb = singles.tile([128, KT, SK], FP16, tag="kT")
            outT_sb = singles.tile([128, KT, SQ], FP16, tag="outT")

            # preload the Act exp table set (2.7us) before it's needed
            warm = singles.tile([1, 16], F32, tag="warm")
            nc.vector.memset(warm, 0.0)
            warm2 = singles.tile([1, 16], F32, tag="warm2")
            nc.scalar.activation(out=warm2, in_=warm,
                                 func=mybir.ActivationFunctionType.Exp)

            # ---- projections ----
            with tc.tile_pool(name="pproj", bufs=4, space="PSUM") as pproj:
                # v projection: per 128-key tile st, needs x chunk st//4 only
                for st in range(NKT):
                    ps = pproj.tile([128, 512], F32, tag="pp")
                    for ke in range(KT):
                        nc.tensor.matmul(
                            ps,
                            xT_sb[:, ke, ts(st, 128)],
                            W_sb["Wv"][:, ke, :],
                            start=(ke == 0), stop=(ke == KT - 1),
                        )
                    nc.scalar.activation(
                        out=v_sb[:, st, :, 0:DH],
                        in_=ps.rearrange("p (h d) -> p h d", h=H),
                        func=mybir.ActivationFunctionType.Copy,
                    )
                # k projection: ke-outer so each W tile is LDW'd once
                for th in range(KT):
                    pks = [pproj.tile([128, 512], F32, tag="pp", name=f"pk{kc}")
                           for kc in range(4)]
                    for ke in range(KT):
                        for kc in range(4):
                            nc.tensor.matmul(
                                pks[kc],
                                W_sb["Wk"][:, ke, ts(th, 128)],
                                xT_sb[:, ke, ts(kc, 512)],
                                start=(ke == 0), stop=(ke == KT - 1),
                            )
                    for kc in range(4):
                        nc.vector.tensor_scalar_add(
                            out=kT_sb[:, th, ts(kc, 512)], in0=pks[kc],
                            scalar1=bk_sb[:, th:th + 1],
                        )
                # q projection
                for th in range(KT):
                    pqs = [pproj.tile([128, 512], F32, tag="pp", name=f"pq{qc}")
                           for qc in range(2)]
                    for ke in range(KT):
                        for qc in range(2):
                            nc.tensor.matmul(
                                pqs[qc],
                                W_sb["Wq"][:, ke, ts(th, 128)],
                                xT_sb[:, ke, HALF + qc * 512:HALF + (qc + 1) * 512],
                                start=(ke == 0), stop=(ke == KT - 1),
                            )
                    for qc in range(2):
                        nc.vector.tensor_scalar_add(
                            out=qT_sb[:, th, ts(qc, 512)], in0=pqs[qc],
                            scalar1=bq_sb[:, th:th + 1],
                        )

            # ---- windowed attention, two heads (one head-pair) at a time ----
            with (
                tc.tile_pool(name="pscore", bufs=4, space="PSUM") as pscore,
                tc.tile_pool(name="pav", bufs=2, space="PSUM") as pav,
            ):
                for hp in range(H // 2):
                    th = hp
                    avps = []
                    for i in range(2):
                        av_i = pav.tile([DH + 1, SQ], F32, tag="av", name=f"av{i}")
                        avps.append(av_i)
                    for kt in range(NKT):
                        # scores: 4x [128,512] tiles; heads interleaved so the
                        # two K=64 matmuls row-pack into array halves
                        s_t = {}
                        for qc in range(2):
                            for i in range(2):
                                r0 = 64 * i
                                sp = pscore.tile([128, 512], F32, tag="s",
                                                 name=f"s{i}q{qc}")
                                nc.tensor.matmul(
                                    sp,
                                    kT_sb[r0:r0 + 64, th, ts(kt, 128)],
                                    qT_sb[r0:r0 + 64, th, ts(qc, 512)],
                                    start=True, stop=True,
                                )
                                s_t[(i, qc)] = sp
                        e_t = {}
                        for qc in range(2):
                            e0 = exps.tile([128, 512], FP16, tag="e", name=f"e0q{qc}")
                            nc.vector._custom_dve(
                                exp_op, out=e0, in0=s_t[(0, qc)],
                                s0=_EC1, s1=_EC2, imm2=_EC3,
                            )
                            e_t[(0, qc)] = e0
                            e1 = exps.tile([128, 512], FP16, tag="e", name=f"e1q{qc}")
                            nc.scalar.activation(
                                out=e1, in_=s_t[(1, qc)],
                                func=mybir.ActivationFunctionType.Exp, scale=SCALE,
                            )
                            e_t[(1, qc)] = e1
                        for qc in range(2):
                            for i in range(2):
                                h = 2 * hp + i
                                nc.tensor.matmul(
                                    avps[i][:, ts(qc, 512)],
                                    v_sb[:, kt, h, :],
                                    e_t[(i, qc)],
                                    start=(kt == 0), stop=(kt == NKT - 1),
                                )
                    # finalize: evacuate accumulators, then normalize on the
                    # SBUF side (GpSimd divide by a DMA-broadcast denominator)
                    for i in range(2):
                        h = 2 * hp + i
                        r0 = 64 * i
                        avu = avus.tile([DH + 1, SQ], F32, tag="avu")
                        nc.scalar.activation(out=avu, in_=avps[i],
                                             func=mybir.ActivationFunctionType.Copy)
                        # denominator -> [64,16] via DRAM (cross-partition
                        # reshape) so the reciprocal runs on 64 DVE lanes,
                        # then back to a DRAM row for the stride-0 partition
                        # broadcast.  Multiplies run on GpSimd in steady state
                        # (keeps the Vector FIFO pure exp); the last head-pair
                        # uses Vector, which is idle by then, to shorten the
                        # tail before the output projection.
                        d_dram = dscratch.tile([1, SQ], F32, tag="dd")
                        nc.sync.dma_start(out=d_dram, in_=avu[DH:DH + 1, :])
                        ds = bcs.tile([DH, SQ // DH], F32, tag="ds")
                        nc.sync.dma_start(
                            out=ds,
                            in_=d_dram.rearrange("o (p f) -> (o p) f", p=DH),
                        )
                        rs = bcs.tile([DH, SQ // DH], F32, tag="rs")
                        nc.vector.reciprocal(out=rs, in_=ds)
                        r_dram = dscratch.tile([1, SQ], F32, tag="rd")
                        nc.sync.dma_start(
                            out=r_dram.rearrange("o (p f) -> (o p) f", p=DH),
                            in_=rs,
                        )
                        bc = bcs.tile([DH, SQ], F32, tag="bc")
                        nc.sync.dma_start(
                            out=bc,
                            in_=bass.AP(
                                tensor=r_dram.tensor,
                                offset=r_dram.offset,
                                ap=[[0, DH]] + [list(a) for a in r_dram.ap[1:]],
                            ),
                        )
                        mul_eng = nc.vector if hp == H // 2 - 1 else nc.gpsimd
                        for qc in range(2):
                            mul_eng.tensor_mul(
                                out=outT_sb[r0:r0 + 64, th, ts(qc, 512)],
                                in0=avu[0:DH, ts(qc, 512)],
                                in1=bc[:, ts(qc, 512)],
                            )

            # ---- output projection ----
            with tc.tile_pool(name="py", bufs=4, space="PSUM") as py:
                for m in range(KT):
                    pys = [py.tile([128, 512], F32, tag="py", name=f"py{qc}")
                           for qc in range(2)]
                    for ke in range(KT):
                        for qc in range(2):
                            nc.tensor.matmul(
                                pys[qc],
                                W_sb["Wo"][:, ke, ts(m, 128)],
                                outT_sb[:, ke, ts(qc, 512)],
                                start=(ke == 0), stop=(ke == KT - 1),
                            )
                    for qc in range(2):
                        yst = ystage.tile([128, 512], FP16, tag="y")
                        nc.vector.tensor_scalar_add(out=yst, in0=pys[qc],
                                                    scalar1=bo_sb[:, m:m + 1])
                        nc.sync.dma_start(out=yT_d[ts(m, 128), ts(qc, 512)], in_=yst)

    nc.compile()
    return nc


_NC_CACHE = []


def _get_nc():
    if not _NC_CACHE:
        _NC_CACHE.append(_build())
    return _NC_CACHE[0]


def _prep_inputs(x, Wq, bq, Wk, bk, Wv, bv, Wo, bo):
    x = np.asarray(x, np.float32)
    xT_full = np.ascontiguousarray(x[0].T)  # [E, S]
    bo_eff = (np.asarray(bo, np.float64)
              + np.asarray(bv, np.float64) @ np.asarray(Wo, np.float64)).astype(np.float32)
    def wprep(W):
        Wb = np.asarray(W, np.float32).astype(np.float16)
        return np.ascontiguousarray(Wb.reshape(4, 128, E).transpose(1, 0, 2))

    shared = {
        "Wq": wprep(Wq),
        "Wk": wprep(Wk),
        "Wv": wprep(Wv),
        "Wo": wprep(Wo),
        "bq": np.asarray(bq, np.float32),
        "bk": np.asarray(bk, np.float32),
        "bo_eff": bo_eff,
    }
    in_maps = []
    for c in range(NCORES):
        g0 = 1024 * c - HALF
        xT_halo = np.zeros((E, SK), np.float32)
        lo, hi = max(0, g0), min(S, g0 + SK)
        xT_halo[:, lo - g0:hi - g0] = xT_full[:, lo:hi]
        mask = np.zeros((SK, H), np.float32)
        mask[lo - g0:hi - g0, :] = 1.0
        mask = np.ascontiguousarray(mask.reshape(SK // 128, 128, H).transpose(1, 0, 2))
        m = dict(shared)
        m["xT"] = xT_halo.astype(np.float16)
        m["mask8"] = mask.astype(np.float16)
        in_maps.append(m)
    return in_maps


def run(inputs: dict, trace: bool = False):
    nc = _get_nc()
    in_maps = _prep_inputs(**inputs)
    res = bass_utils.run_bass_kernel_spmd(
        nc, in_maps, core_ids=list(range(NCORES)), trace=trace
    )
    y = np.concatenate([r["yT"].T for r in res.results], axis=0)[None]
    return np.ascontiguousarray(y.astype(np.float32)), res


def kernel(**inputs) -> np.ndarray:
    y, _ = run(inputs, trace=False)
    return y


# revision 15
# speedup vs baseline: 1.3085x; 1.0183x over previous
"""Sliding-window (chunked) multi-head attention for Trainium2, 8-core SPMD.

Problem: B=1, S=8192, E=512, H=8 heads, Dh=64, window=1024 (half=512).
Reference math per window i (size 1024): keys span [i-512, i+1536).

Sharding: core c owns query window [1024c, 1024c+1024); it receives
x^T for the halo'd key range [1024c-512, 1024c+1536) (zero-padded at
the sequence edges) and computes q/k/v projections locally, windowed
softmax(q k^T / 8) v, and the output projection.  All compute layouts
are transposed ([E, seq]) so every matmul contracts over partitions;
the softmax denominator comes from a ones-augmented v (65th column).
bv is folded into an adjusted output-projection bias on the host
(attn rows sum to 1), so v needs no bias add on-chip.

Pipeline structure (v2):
 - x is DMA'd in four 512-column sequence chunks on four DMA rings so
   the v projection starts ~1.5us in.
 - PSUM: 4x [128,512] score tiles (1 bank each) + 2x [65,1024] AV
   accumulators (2 banks each) = 8 banks, double-buffered end to end.
 - exp split: head0 tiles on Vector (custom cubic^4 DVE op), head1
   tiles on Scalar (Act exp).  All PSUM evacuations live on V/S
   (GpSimd has no PSUM port on TRN2); GpSimd does the SBUF-side
   normalize via tensor_tensor divide on a DMA-broadcast denominator.
 - score matmuls for the two heads are issued interleaved so their
   K=64 matmuls row-pack into disjoint halves of the PE array.

Outputs are y^T shards [512, 1024] per core; the host transposes and
concatenates.
"""

import numpy as np
import ml_dtypes

import concourse.bass as bass
import concourse.tile as tile
from concourse import bacc, mybir
from concourse import bass_utils
from concourse.bass import ts

# ---- problem constants (hardcoded per contract) ----
S = 8192
E = 512
H = 8
DH = 64
NCORES = 8
SQ = 1024          # queries per core
SK = 2048          # halo'd keys per core
HALF = 512
SCALE = 0.125      # 1/sqrt(64)

F32 = mybir.dt.float32
F32R = mybir.dt.float32r
BF16 = mybir.dt.bfloat16
FP16 = mybir.dt.float16

# ---- custom DVE op: exp(u/8) ~= (1 + c1 u + c2 u^2 + c3 u^3)^4 ----
# Fitted (Lawson minimax) on |u/8| <= 1.6; max rel err 7.2e-4.
_EC1 = 0.03126080224663743
_EC2 = 0.000493647595612354
_EC3 = 5.0261583805949835e-06


def _register_exp_op():
    from concourse import dve_ops as dops
    from concourse.dve_spec import Spec, Src0, One, C0, C1, C2, sq, lower
    from concourse.dve_uop import DveOpSpec

    name = "EXP4_ANT"
    for op in dops.OPS:
        if op.name == name:
            return op
    body = sq(sq(((C2 * Src0 + C1) * Src0 + C0) * Src0 + One))
    spec = Spec(body=body)
    shas = {}
    for ver in ("v3", "v4"):
        uops = lower(spec, ver=ver)
        shas[ver] = DveOpSpec(name=name, opcode=0, uops=uops, rd1_en=False).sha(ver)
    op = dops.DveOp(name, spec, subdim=False, uops_sha=shas)
    dops.OPS.append(op)
    dops.CUSTOM_DVE_SPECS[name] = spec
    dops._SUB_OPCODE_FOR_NAME[name] = dops._CUSTOM_DVE_ROW_BASE + len(dops.OPS) - 1
    assert max(dops._SUB_OPCODE_FOR_NAME.values()) < 0x20
    return op


def _build():
    """Build + compile the per-core Bass program (SPMD: same NEFF, 8 cores)."""
    exp_op = _register_exp_op()

    nc = bacc.Bacc("TRN2", target_bir_lowering=False, debug=False)

    xT_d = nc.dram_tensor("xT", [E, SK], FP16, kind="ExternalInput")
    W_d = {
        n: nc.dram_tensor(n, [128, E // 128, E], FP16, kind="ExternalInput")
        for n in ("Wq", "Wk", "Wv", "Wo")
    }
    bq_d = nc.dram_tensor("bq", [E], F32, kind="ExternalInput")
    bk_d = nc.dram_tensor("bk", [E], F32, kind="ExternalInput")
    bo_d = nc.dram_tensor("bo_eff", [E], F32, kind="ExternalInput")
    mask_d = nc.dram_tensor("mask8", [128, SK // 128, H], FP16, kind="ExternalInput")
    yT_d = nc.dram_tensor("yT", [E, SQ], FP16, kind="ExternalOutput")

    KT = 4           # E // 128 contraction tiles
    NKT = SK // 128  # 16 key tiles

    with tile.TileContext(nc) as tc:
        with (
            nc.allow_low_precision(reason="fp16/f32r attention kernel"),
            tc.tile_pool(name="singles", bufs=1) as singles,
            tc.tile_pool(name="exps", bufs=8) as exps,
            tc.tile_pool(name="avus", bufs=3) as avus,
            tc.tile_pool(name="bcs", bufs=4) as bcs,
            tc.tile_pool(name="dscratch", bufs=2, space="DRAM") as dscratch,
            tc.tile_pool(name="ystage", bufs=3) as ystage,
        ):
            # ---- input DMAs: x in 4 seq-chunks on 4 rings ----
            W_sb = {}
            for n in ("Wq", "Wk", "Wv", "Wo"):
                W_sb[n] = singles.tile([128, KT, E], FP16, tag=f"w_{n}", name=f"w_{n}")
            xT_sb = singles.tile([128, KT, SK], FP16, tag="xT")

            xin = xT_d.ap().rearrange("(t p) s -> p t s", p=128)
            nc.sync.dma_start(out=W_sb["Wv"], in_=W_d["Wv"].ap())
            nc.scalar.dma_start(out=xT_sb[:, :, ts(0, 512)], in_=xin[:, :, ts(0, 512)])
            nc.gpsimd.dma_start(out=xT_sb[:, :, ts(1, 512)], in_=xin[:, :, ts(1, 512)])
            nc.sync.dma_start(out=xT_sb[:, :, ts(2, 512)], in_=xin[:, :, ts(2, 512)])
            nc.scalar.dma_start(out=xT_sb[:, :, ts(3, 512)], in_=xin[:, :, ts(3, 512)])
            nc.gpsimd.dma_start(out=W_sb["Wq"], in_=W_d["Wq"].ap())
            nc.scalar.dma_start(out=W_sb["Wk"], in_=W_d["Wk"].ap())
            nc.gpsimd.dma_start(out=W_sb["Wo"], in_=W_d["Wo"].ap())

            bq_sb = singles.tile([128, KT], F32, tag="bq")
            nc.sync.dma_start(out=bq_sb, in_=bq_d.ap().rearrange("(t p) -> p t", p=128))
            bk_sb = singles.tile([128, KT], F32, tag="bk")
            nc.sync.dma_start(out=bk_sb, in_=bk_d.ap().rearrange("(t p) -> p t", p=128))
            bo_sb = singles.tile([128, KT], F32, tag="bo")
            nc.sync.dma_start(out=bo_sb, in_=bo_d.ap().rearrange("(t p) -> p t", p=128))

            # v with ones column (from mask: 0 for padded keys)
            v_sb = singles.tile([128, NKT, H, DH + 1], FP16, tag="v")
            nc.sync.dma_start(out=v_sb[:, :, :, DH], in_=mask_d.ap())

            qT_sb = singles.tile([128, KT, SQ], FP16, tag="qT")
            kT_sb = singles.tile([128, KT, SK], FP16, tag="kT")
            outT_sb = singles.tile([128, KT, SQ], FP16, tag="outT")

            # preload the Act exp table set (2.7us) before it's needed
            warm = singles.tile([1, 16], F32, tag="warm")
            nc.vector.memset(warm, 0.0)
            warm2 = singles.tile([1, 16], F32, tag="warm2")
            nc.scalar.activation(out=warm2, in_=warm,
                                 func=mybir.ActivationFunctionType.Exp)

            # ---- projections ----
            with tc.tile_pool(name="pproj", bufs=4, space="PSUM") as pproj:
                # v projection: per 128-key tile st, needs x chunk st//4 only
                for st in range(NKT):
                    ps = pproj.tile([128, 512], F32, tag="pp")
                    for ke in range(KT):
                        nc.tensor.matmul(
                            ps,
                            xT_sb[:, ke, ts(st, 128)],
                            W_sb["Wv"][:, ke, :],
                            start=(ke == 0), stop=(ke == KT - 1),
                        )
                    nc.scalar.activation(
                        out=v_sb[:, st, :, 0:DH],
                        in_=ps.rearrange("p (h d) -> p h d", h=H),
                        func=mybir.ActivationFunctionType.Copy,
                    )
                # k projection: ke-outer so each W tile is LDW'd once
                for th in range(KT):
                    pks = [pproj.tile([128, 512], F32, tag="pp", name=f"pk{kc}")
                           for kc in range(4)]
                    for ke in range(KT):
                        for kc in range(4):
                            nc.tensor.matmul(
                                pks[kc],
                                W_sb["Wk"][:, ke, ts(th, 128)],
                                xT_sb[:, ke, ts(kc, 512)],
                                start=(ke == 0), stop=(ke == KT - 1),
                            )
                    for kc in range(4):
                        nc.vector.tensor_scalar_add(
                            out=kT_sb[:, th, ts(kc, 512)], in0=pks[kc],
                            scalar1=bk_sb[:, th:th + 1],
                        )
                # q projection
                for th in range(KT):
                    pqs = [pproj.tile([128, 512], F32, tag="pp", name=f"pq{qc}")
                           for qc in range(2)]
                    for ke in range(KT):
                        for qc in range(2):
                            nc.tensor.matmul(
                                pqs[qc],
                                W_sb["Wq"][:, ke, ts(th, 128)],
                                xT_sb[:, ke, HALF + qc * 512:HALF + (qc + 1) * 512],
                                start=(ke == 0), stop=(ke == KT - 1),
                            )
                    for qc in range(2):
                        nc.vector.tensor_scalar_add(
                            out=qT_sb[:, th, ts(qc, 512)], in0=pqs[qc],
                            scalar1=bq_sb[:, th:th + 1],
                        )

            # ---- windowed attention, two heads (one head-pair) at a time ----
            with (
                tc.tile_pool(name="pscore", bufs=4, space="PSUM") as pscore,
                tc.tile_pool(name="pav", bufs=2, space="PSUM") as pav,
            ):
                for hp in range(H // 2):
                    th = hp
                    avps = []
                    for i in range(2):
                        av_i = pav.tile([DH + 1, SQ], F32, tag="av", name=f"av{i}")
                        avps.append(av_i)
                    for kt in range(NKT):
                        # scores: 4x [128,512] tiles; heads interleaved so the
                        # two K=64 matmuls row-pack into array halves
                        s_t = {}
                        for qc in range(2):
                            for i in range(2):
                                r0 = 64 * i
                                sp = pscore.tile([128, 512], F32, tag="s",
                                                 name=f"s{i}q{qc}")
                                nc.tensor.matmul(
                                    sp,
                                    kT_sb[r0:r0 + 64, th, ts(kt, 128)],
                                    qT_sb[r0:r0 + 64, th, ts(qc, 512)],
                                    start=True, stop=True,
                                )
                                s_t[(i, qc)] = sp
                        e_t = {}
                        for qc in range(2):
                            e0 = exps.tile([128, 512], FP16, tag="e", name=f"e0q{qc}")
                            nc.vector._custom_dve(
                                exp_op, out=e0, in0=s_t[(0, qc)],
                                s0=_EC1, s1=_EC2, imm2=_EC3,
                            )
                            e_t[(0, qc)] = e0
                            e1 = exps.tile([128, 512], FP16, tag="e", name=f"e1q{qc}")
                            nc.scalar.activation(
                                out=e1, in_=s_t[(1, qc)],
                                func=mybir.ActivationFunctionType.Exp, scale=SCALE,
                            )
                            e_t[(1, qc)] = e1
                        for qc in range(2):
                            for i in range(2):
                                h = 2 * hp + i
                                nc.tensor.matmul(
                                    avps[i][:, ts(qc, 512)],
                                    v_sb[:, kt, h, :],
                                    e_t[(i, qc)],
                                    start=(kt == 0), stop=(kt == NKT - 1),
                                )
                    # finalize: evacuate accumulators, then normalize on the
                    # SBUF side (GpSimd divide by a DMA-broadcast denominator)
                    for i in range(2):
                        h = 2 * hp + i
                        r0 = 64 * i
                        avu = avus.tile([DH + 1, SQ], F32, tag="avu")
                        nc.scalar.activation(out=avu, in_=avps[i],
                                             func=mybir.ActivationFunctionType.Copy)
                        # denominator -> [64,16] via DRAM (cross-partition
                        # reshape) so the reciprocal runs on 64 DVE lanes,
                        # then back to a DRAM row for the stride-0 partition
                        # broadcast.  Multiplies run on GpSimd in steady state
                        # (keeps the Vector FIFO pure exp); the last head-pair
                        # uses Vector, which is idle by then, to shorten the
                        # tail before the output projection.
                        d_dram = dscratch.tile([1, SQ], F32, tag="dd")
                        nc.sync.dma_start(out=d_dram, in_=avu[DH:DH + 1, :])
                        ds = bcs.tile([DH, SQ // DH], F32, tag="ds")
                        nc.sync.dma_start(
                            out=ds,
                            in_=d_dram.rearrange("o (p f) -> (o p) f", p=DH),
                        )
                        rs = bcs.tile([DH, SQ // DH], F32, tag="rs")
                        nc.vector.reciprocal(out=rs, in_=ds)
                        r_dram = dscratch.tile([1, SQ], F32, tag="rd")
                        nc.sync.dma_start(
                            out=r_dram.rearrange("o (p f) -> (o p) f", p=DH),
                            in_=rs,
                        )
                        mul_eng = nc.vector if hp == H // 2 - 1 else nc.gpsimd
                        for qc in range(2):
                            # per-half broadcast so the first multiply starts
                            # as soon as half the (slow, stride-0) DMA lands
                            bch = bcs.tile([DH, 512], F32, tag="bc",
                                           name=f"bc{qc}")
                            rsrc = r_dram[:, ts(qc, 512)]
                            nc.sync.dma_start(
                                out=bch,
                                in_=bass.AP(
                                    tensor=rsrc.tensor,
                                    offset=rsrc.offset,
                                    ap=[[0, DH]] + [list(a) for a in rsrc.ap[1:]],
                                ),
                            )
                            mul_eng.tensor_mul(
                                out=outT_sb[r0:r0 + 64, th, ts(qc, 512)],
                                in0=avu[0:DH, ts(qc, 512)],
                                in1=bch,
                            )

            # ---- output projection ----
            with tc.tile_pool(name="py", bufs=4, space="PSUM") as py:
                for m in range(KT):
                    pys = [py.tile([128, 512], F32, tag="py", name=f"py{qc}")
                           for qc in range(2)]
                    for ke in range(KT):
                        for qc in range(2):
                            nc.tensor.matmul(
                                pys[qc],
                                W_sb["Wo"][:, ke, ts(m, 128)],
                                outT_sb[:, ke, ts(qc, 512)],
                                start=(ke == 0), stop=(ke == KT - 1),
                            )
                    for qc in range(2):
                        yst = ystage.tile([128, 512], FP16, tag="y")
                        nc.vector.tensor_scalar_add(out=yst, in0=pys[qc],
                                                    scalar1=bo_sb[:, m:m + 1])
                        nc.sync.dma_start(out=yT_d[ts(m, 128), ts(qc, 512)], in_=yst)

    nc.compile()
    return nc


_NC_CACHE = []


def _get_nc():
    if not _NC_CACHE:
        _NC_CACHE.append(_build())
    return _NC_CACHE[0]


def _prep_inputs(x, Wq, bq, Wk, bk, Wv, bv, Wo, bo):
    x = np.asarray(x, np.float32)
    xT_full = np.ascontiguousarray(x[0].T)  # [E, S]
    bo_eff = (np.asarray(bo, np.float64)
              + np.asarray(bv, np.float64) @ np.asarray(Wo, np.float64)).astype(np.float32)
    def wprep(W):
        Wb = np.asarray(W, np.float32).astype(np.float16)
        return np.ascontiguousarray(Wb.reshape(4, 128, E).transpose(1, 0, 2))

    shared = {
        "Wq": wprep(Wq),
        "Wk": wprep(Wk),
        "Wv": wprep(Wv),
        "Wo": wprep(Wo),
        "bq": np.asarray(bq, np.float32),
        "bk": np.asarray(bk, np.float32),
        "bo_eff": bo_eff,
    }
    in_maps = []
    for c in range(NCORES):
        g0 = 1024 * c - HALF
        xT_halo = np.zeros((E, SK), np.float32)
        lo, hi = max(0, g0), min(S, g0 + SK)
        xT_halo[:, lo - g0:hi - g0] = xT_full[:, lo:hi]
        mask = np.zeros((SK, H), np.float32)
        mask[lo - g0:hi - g0, :] = 1.0
        mask = np.ascontiguousarray(mask.reshape(SK // 128, 128, H).transpose(1, 0, 2))
        m = dict(shared)
        m["xT"] = xT_halo.astype(np.float16)
        m["mask8"] = mask.astype(np.float16)
        in_maps.append(m)
    return in_maps


def run(inputs: dict, trace: bool = False):
    nc = _get_nc()
    in_maps = _prep_inputs(**inputs)
    res = bass_utils.run_bass_kernel_spmd(
        nc, in_maps, core_ids=list(range(NCORES)), trace=trace
    )
    y = np.concatenate([r["yT"].T for r in res.results], axis=0)[None]
    return np.ascontiguousarray(y.astype(np.float32)), res


def kernel(**inputs) -> np.ndarray:
    y, _ = run(inputs, trace=False)
    return y


# revision 17
# speedup vs baseline: 1.3821x; 1.0563x over previous
"""Sliding-window (chunked) multi-head attention for Trainium2, 8-core SPMD.

Problem: B=1, S=8192, E=512, H=8 heads, Dh=64, window=1024 (half=512).
Reference math per window i (size 1024): keys span [i-512, i+1536).

Sharding: core c owns query window [1024c, 1024c+1024); it receives
x^T for the halo'd key range [1024c-512, 1024c+1536) (zero-padded at
the sequence edges) and computes q/k/v projections locally, windowed
softmax(q k^T / 8) v, and the output projection.  All compute layouts
are transposed ([E, seq]) so every matmul contracts over partitions;
the softmax denominator comes from a ones-augmented v (65th column).
bv is folded into an adjusted output-projection bias on the host
(attn rows sum to 1), so v needs no bias add on-chip.

Pipeline structure (v2):
 - x is DMA'd in four 512-column sequence chunks on four DMA rings so
   the v projection starts ~1.5us in.
 - PSUM: 4x [128,512] score tiles (1 bank each) + 2x [65,1024] AV
   accumulators (2 banks each) = 8 banks, double-buffered end to end.
 - exp split: head0 tiles on Vector (custom cubic^4 DVE op), head1
   tiles on Scalar (Act exp).  All PSUM evacuations live on V/S
   (GpSimd has no PSUM port on TRN2); GpSimd does the SBUF-side
   normalize via tensor_tensor divide on a DMA-broadcast denominator.
 - score matmuls for the two heads are issued interleaved so their
   K=64 matmuls row-pack into disjoint halves of the PE array.

Outputs are y^T shards [512, 1024] per core; the host transposes and
concatenates.
"""

import numpy as np
import ml_dtypes

import concourse.bass as bass
import concourse.tile as tile
from concourse import bacc, mybir
from concourse import bass_utils
from concourse.bass import ts

# ---- problem constants (hardcoded per contract) ----
S = 8192
E = 512
H = 8
DH = 64
NCORES = 8
SQ = 1024          # queries per core
SK = 2048          # halo'd keys per core
HALF = 512
SCALE = 0.125      # 1/sqrt(64)

F32 = mybir.dt.float32
F32R = mybir.dt.float32r
BF16 = mybir.dt.bfloat16
FP16 = mybir.dt.float16
F8 = mybir.dt.float8e4

# q/k weights and x are fp8 with W pre-scaled by 64 on the host (e4m3's
# normal range starts at 2^-6), so raw scores are 4096x the true scores;
# the exp scale absorbs it.  Wv is also 64x, folded back via Wo/64.
WSCALE = 64.0
SSCALE = WSCALE * WSCALE

# ---- custom DVE op: exp(u/8) ~= (1 + c1 u + c2 u^2 + c3 u^3)^4 ----
# Fitted (Lawson minimax) on |u/8| <= 1.6; max rel err 7.2e-4.
_EC1 = 0.03126080224663743
_EC2 = 0.000493647595612354
_EC3 = 5.0261583805949835e-06


def _register_exp_op():
    from concourse import dve_ops as dops
    from concourse.dve_spec import Spec, Src0, One, C0, C1, C2, sq, lower
    from concourse.dve_uop import DveOpSpec

    name = "EXP4_ANT"
    for op in dops.OPS:
        if op.name == name:
            return op
    body = sq(sq(((C2 * Src0 + C1) * Src0 + C0) * Src0 + One))
    spec = Spec(body=body)
    shas = {}
    for ver in ("v3", "v4"):
        uops = lower(spec, ver=ver)
        shas[ver] = DveOpSpec(name=name, opcode=0, uops=uops, rd1_en=False).sha(ver)
    op = dops.DveOp(name, spec, subdim=False, uops_sha=shas)
    dops.OPS.append(op)
    dops.CUSTOM_DVE_SPECS[name] = spec
    dops._SUB_OPCODE_FOR_NAME[name] = dops._CUSTOM_DVE_ROW_BASE + len(dops.OPS) - 1
    assert max(dops._SUB_OPCODE_FOR_NAME.values()) < 0x20
    return op


def _build():
    """Build + compile the per-core Bass program (SPMD: same NEFF, 8 cores)."""
    exp_op = _register_exp_op()

    nc = bacc.Bacc("TRN2", target_bir_lowering=False, debug=False)

    xT_d = nc.dram_tensor("xT", [E, SK], F8, kind="ExternalInput")
    W_d = {
        n: nc.dram_tensor(n, [128, E // 128, E], F8 if n != "Wo" else FP16,
                          kind="ExternalInput")
        for n in ("Wq", "Wk", "Wv", "Wo")
    }
    bq_d = nc.dram_tensor("bq", [E], F32, kind="ExternalInput")
    bk_d = nc.dram_tensor("bk", [E], F32, kind="ExternalInput")
    bo_d = nc.dram_tensor("bo_eff", [E], F32, kind="ExternalInput")
    mask_d = nc.dram_tensor("mask8", [128, SK // 128, H], FP16, kind="ExternalInput")
    yT_d = nc.dram_tensor("yT", [E, SQ], FP16, kind="ExternalOutput")

    KT = 4           # E // 128 contraction tiles
    NKT = SK // 128  # 16 key tiles

    with tile.TileContext(nc) as tc:
        with (
            nc.allow_low_precision(reason="fp16/f32r attention kernel"),
            tc.tile_pool(name="singles", bufs=1) as singles,
            tc.tile_pool(name="exps", bufs=8) as exps,
            tc.tile_pool(name="avus", bufs=3) as avus,
            tc.tile_pool(name="bcs", bufs=4) as bcs,
            tc.tile_pool(name="dscratch", bufs=2, space="DRAM") as dscratch,
            tc.tile_pool(name="ystage", bufs=3) as ystage,
        ):
            # ---- input DMAs: x in 4 seq-chunks on 4 rings ----
            W_sb = {}
            for n in ("Wq", "Wk", "Wv", "Wo"):
                W_sb[n] = singles.tile([128, KT, E], F8 if n != "Wo" else FP16,
                                       tag=f"w_{n}", name=f"w_{n}")
            xT_sb = singles.tile([128, KT, SK], F8, tag="xT")

            xin = xT_d.ap().rearrange("(t p) s -> p t s", p=128)
            nc.sync.dma_start(out=W_sb["Wv"], in_=W_d["Wv"].ap())
            nc.scalar.dma_start(out=xT_sb[:, :, ts(0, 512)], in_=xin[:, :, ts(0, 512)])
            nc.gpsimd.dma_start(out=xT_sb[:, :, ts(1, 512)], in_=xin[:, :, ts(1, 512)])
            nc.sync.dma_start(out=xT_sb[:, :, ts(2, 512)], in_=xin[:, :, ts(2, 512)])
            nc.scalar.dma_start(out=xT_sb[:, :, ts(3, 512)], in_=xin[:, :, ts(3, 512)])
            nc.gpsimd.dma_start(out=W_sb["Wq"], in_=W_d["Wq"].ap())
            nc.scalar.dma_start(out=W_sb["Wk"], in_=W_d["Wk"].ap())
            nc.gpsimd.dma_start(out=W_sb["Wo"], in_=W_d["Wo"].ap())

            bq_sb = singles.tile([128, KT], F32, tag="bq")
            nc.sync.dma_start(out=bq_sb, in_=bq_d.ap().rearrange("(t p) -> p t", p=128))
            bk_sb = singles.tile([128, KT], F32, tag="bk")
            nc.sync.dma_start(out=bk_sb, in_=bk_d.ap().rearrange("(t p) -> p t", p=128))
            bo_sb = singles.tile([128, KT], F32, tag="bo")
            nc.sync.dma_start(out=bo_sb, in_=bo_d.ap().rearrange("(t p) -> p t", p=128))

            # v with ones column (from mask: 0 for padded keys)
            v_sb = singles.tile([128, NKT, H, DH + 1], FP16, tag="v")
            nc.sync.dma_start(out=v_sb[:, :, :, DH], in_=mask_d.ap())

            qT_sb = singles.tile([128, KT, SQ], FP16, tag="qT")
            kT_sb = singles.tile([128, KT, SK], FP16, tag="kT")
            outT_sb = singles.tile([128, KT, SQ], FP16, tag="outT")

            # preload the Act exp table set (2.7us) before it's needed
            warm = singles.tile([1, 16], F32, tag="warm")
            nc.vector.memset(warm, 0.0)
            warm2 = singles.tile([1, 16], F32, tag="warm2")
            nc.scalar.activation(out=warm2, in_=warm,
                                 func=mybir.ActivationFunctionType.Exp)

            # ---- projections ----
            with tc.tile_pool(name="pproj", bufs=4, space="PSUM") as pproj:
                # v projection: per 128-key tile st, needs x chunk st//4 only
                for st in range(NKT):
                    ps = pproj.tile([128, 512], F32, tag="pp")
                    for kp in range(KT // 2):
                        nc.tensor.matmul(
                            ps,
                            xT_sb[:, 2 * kp:2 * kp + 2, ts(st, 128)],
                            W_sb["Wv"][:, 2 * kp:2 * kp + 2, :],
                            start=(kp == 0), stop=(kp == KT // 2 - 1),
                            perf_mode=mybir.MatmulPerfMode.DoubleRow,
                        )
                    nc.scalar.activation(
                        out=v_sb[:, st, :, 0:DH],
                        in_=ps.rearrange("p (h d) -> p h d", h=H),
                        func=mybir.ActivationFunctionType.Copy,
                    )
                # k projection: ke-outer so each W tile is LDW'd once
                for th in range(KT):
                    pks = [pproj.tile([128, 512], F32, tag="pp", name=f"pk{kc}")
                           for kc in range(4)]
                    for kp in range(KT // 2):
                        for kc in range(4):
                            nc.tensor.matmul(
                                pks[kc],
                                W_sb["Wk"][:, 2 * kp:2 * kp + 2, ts(th, 128)],
                                xT_sb[:, 2 * kp:2 * kp + 2, ts(kc, 512)],
                                start=(kp == 0), stop=(kp == KT // 2 - 1),
                                perf_mode=mybir.MatmulPerfMode.DoubleRow,
                            )
                    for kc in range(4):
                        nc.vector.tensor_scalar_add(
                            out=kT_sb[:, th, ts(kc, 512)], in0=pks[kc],
                            scalar1=bk_sb[:, th:th + 1],
                        )
                # q projection
                for th in range(KT):
                    pqs = [pproj.tile([128, 512], F32, tag="pp", name=f"pq{qc}")
                           for qc in range(2)]
                    for kp in range(KT // 2):
                        for qc in range(2):
                            nc.tensor.matmul(
                                pqs[qc],
                                W_sb["Wq"][:, 2 * kp:2 * kp + 2, ts(th, 128)],
                                xT_sb[:, 2 * kp:2 * kp + 2,
                                      HALF + qc * 512:HALF + (qc + 1) * 512],
                                start=(kp == 0), stop=(kp == KT // 2 - 1),
                                perf_mode=mybir.MatmulPerfMode.DoubleRow,
                            )
                    for qc in range(2):
                        nc.vector.tensor_scalar_add(
                            out=qT_sb[:, th, ts(qc, 512)], in0=pqs[qc],
                            scalar1=bq_sb[:, th:th + 1],
                        )

            # ---- windowed attention, two heads (one head-pair) at a time ----
            with (
                tc.tile_pool(name="pscore", bufs=4, space="PSUM") as pscore,
                tc.tile_pool(name="pav", bufs=2, space="PSUM") as pav,
            ):
                for hp in range(H // 2):
                    th = hp
                    avps = []
                    for i in range(2):
                        av_i = pav.tile([DH + 1, SQ], F32, tag="av", name=f"av{i}")
                        avps.append(av_i)
                    for kt in range(NKT):
                        # scores: 4x [128,512] tiles; heads interleaved so the
                        # two K=64 matmuls row-pack into array halves
                        s_t = {}
                        for qc in range(2):
                            for i in range(2):
                                r0 = 64 * i
                                sp = pscore.tile([128, 512], F32, tag="s",
                                                 name=f"s{i}q{qc}")
                                nc.tensor.matmul(
                                    sp,
                                    kT_sb[r0:r0 + 64, th, ts(kt, 128)],
                                    qT_sb[r0:r0 + 64, th, ts(qc, 512)],
                                    start=True, stop=True,
                                )
                                s_t[(i, qc)] = sp
                        e_t = {}
                        for qc in range(2):
                            e0 = exps.tile([128, 512], FP16, tag="e", name=f"e0q{qc}")
                            nc.vector._custom_dve(
                                exp_op, out=e0, in0=s_t[(0, qc)],
                                s0=_EC1 / SSCALE, s1=_EC2 / SSCALE**2,
                                imm2=_EC3 / SSCALE**3,
                            )
                            e_t[(0, qc)] = e0
                            e1 = exps.tile([128, 512], FP16, tag="e", name=f"e1q{qc}")
                            nc.scalar.activation(
                                out=e1, in_=s_t[(1, qc)],
                                func=mybir.ActivationFunctionType.Exp,
                                scale=SCALE / SSCALE,
                            )
                            e_t[(1, qc)] = e1
                        for qc in range(2):
                            for i in range(2):
                                h = 2 * hp + i
                                nc.tensor.matmul(
                                    avps[i][:, ts(qc, 512)],
                                    v_sb[:, kt, h, :],
                                    e_t[(i, qc)],
                                    start=(kt == 0), stop=(kt == NKT - 1),
                                )
                    # finalize: evacuate accumulators, then normalize on the
                    # SBUF side (GpSimd divide by a DMA-broadcast denominator)
                    for i in range(2):
                        h = 2 * hp + i
                        r0 = 64 * i
                        avu = avus.tile([DH + 1, SQ], F32, tag="avu")
                        nc.scalar.activation(out=avu, in_=avps[i],
                                             func=mybir.ActivationFunctionType.Copy)
                        # denominator -> [64,16] via DRAM (cross-partition
                        # reshape) so the reciprocal runs on 64 DVE lanes,
                        # then back to a DRAM row for the stride-0 partition
                        # broadcast.  Multiplies run on GpSimd in steady state
                        # (keeps the Vector FIFO pure exp); the last head-pair
                        # uses Vector, which is idle by then, to shorten the
                        # tail before the output projection.
                        d_dram = dscratch.tile([1, SQ], F32, tag="dd")
                        nc.sync.dma_start(out=d_dram, in_=avu[DH:DH + 1, :])
                        ds = bcs.tile([DH, SQ // DH], F32, tag="ds")
                        nc.sync.dma_start(
                            out=ds,
                            in_=d_dram.rearrange("o (p f) -> (o p) f", p=DH),
                        )
                        rs = bcs.tile([DH, SQ // DH], F32, tag="rs")
                        nc.vector.reciprocal(out=rs, in_=ds)
                        r_dram = dscratch.tile([1, SQ], F32, tag="rd")
                        nc.sync.dma_start(
                            out=r_dram.rearrange("o (p f) -> (o p) f", p=DH),
                            in_=rs,
                        )
                        mul_eng = nc.vector if hp == H // 2 - 1 else nc.gpsimd
                        for qc in range(2):
                            # per-half broadcast so the first multiply starts
                            # as soon as half the (slow, stride-0) DMA lands
                            bch = bcs.tile([DH, 512], F32, tag="bc",
                                           name=f"bc{qc}")
                            rsrc = r_dram[:, ts(qc, 512)]
                            nc.sync.dma_start(
                                out=bch,
                                in_=bass.AP(
                                    tensor=rsrc.tensor,
                                    offset=rsrc.offset,
                                    ap=[[0, DH]] + [list(a) for a in rsrc.ap[1:]],
                                ),
                            )
                            mul_eng.tensor_mul(
                                out=outT_sb[r0:r0 + 64, th, ts(qc, 512)],
                                in0=avu[0:DH, ts(qc, 512)],
                                in1=bch,
                            )

            # ---- output projection ----
            with tc.tile_pool(name="py", bufs=4, space="PSUM") as py:
                for m in range(KT):
                    pys = [py.tile([128, 512], F32, tag="py", name=f"py{qc}")
                           for qc in range(2)]
                    for ke in range(KT):
                        for qc in range(2):
                            nc.tensor.matmul(
                                pys[qc],
                                W_sb["Wo"][:, ke, ts(m, 128)],
                                outT_sb[:, ke, ts(qc, 512)],
                                start=(ke == 0), stop=(ke == KT - 1),
                            )
                    for qc in range(2):
                        yst = ystage.tile([128, 512], FP16, tag="y")
                        nc.vector.tensor_scalar_add(out=yst, in0=pys[qc],
                                                    scalar1=bo_sb[:, m:m + 1])
                        nc.sync.dma_start(out=yT_d[ts(m, 128), ts(qc, 512)], in_=yst)

    nc.compile()
    return nc


_NC_CACHE = []


def _get_nc():
    if not _NC_CACHE:
        _NC_CACHE.append(_build())
    return _NC_CACHE[0]


def _prep_inputs(x, Wq, bq, Wk, bk, Wv, bv, Wo, bo):
    x = np.asarray(x, np.float32)
    xT_full = np.ascontiguousarray(x[0].T)  # [E, S]
    bo_eff = (np.asarray(bo, np.float64)
              + np.asarray(bv, np.float64) @ np.asarray(Wo, np.float64)).astype(np.float32)
    def wlay(Wb):
        return np.ascontiguousarray(Wb.reshape(4, 128, E).transpose(1, 0, 2))

    def wprep8(W):
        Wb = (np.asarray(W, np.float32) * WSCALE).astype(ml_dtypes.float8_e4m3fn)
        return wlay(Wb)

    shared = {
        "Wq": wprep8(Wq),
        "Wk": wprep8(Wk),
        "Wv": wprep8(Wv),
        "Wo": wlay((np.asarray(Wo, np.float32) / WSCALE).astype(np.float16)),
        "bq": np.asarray(bq, np.float32) * WSCALE,
        "bk": np.asarray(bk, np.float32) * WSCALE,
        "bo_eff": bo_eff,
    }
    in_maps = []
    for c in range(NCORES):
        g0 = 1024 * c - HALF
        xT_halo = np.zeros((E, SK), np.float32)
        lo, hi = max(0, g0), min(S, g0 + SK)
        xT_halo[:, lo - g0:hi - g0] = xT_full[:, lo:hi]
        mask = np.zeros((SK, H), np.float32)
        mask[lo - g0:hi - g0, :] = 1.0
        mask = np.ascontiguousarray(mask.reshape(SK // 128, 128, H).transpose(1, 0, 2))
        m = dict(shared)
        m["xT"] = xT_halo.astype(ml_dtypes.float8_e4m3fn)
        m["mask8"] = mask.astype(np.float16)
        in_maps.append(m)
    return in_maps


def run(inputs: dict, trace: bool = False):
    nc = _get_nc()
    in_maps = _prep_inputs(**inputs)
    res = bass_utils.run_bass_kernel_spmd(
        nc, in_maps, core_ids=list(range(NCORES)), trace=trace
    )
    y = np.concatenate([r["yT"].T for r in res.results], axis=0)[None]
    return np.ascontiguousarray(y.astype(np.float32)), res


def kernel(**inputs) -> np.ndarray:
    y, _ = run(inputs, trace=False)
    return y
